# revision 60
# baseline (speedup 1.0000x reference)
"""Trainium2 Bass kernel for nn_AttentionS2 (spherical self-attention).

Module: y = p_w @ softmax_k(q k^T / sqrt(hd) + log_quad_w[k]) v + p_b
with q/k/v = 1x1-conv projections of the same input (self-attention),
B=1, C=512, H=W=64 (4096 tokens), 8 heads, head_dim=64.

Sharding: one head per NeuronCore (8 cores). Each core:
  1. projects q_h, k_h (channel-major) and v_h^T (token-major) for its head;
     all matmul inputs are bf16 (PSUM accumulation stays fp32)
  2. computes S^T = k_h^T q_h in (key x query) orientation, 128-key tiles
  3. exp(scale*S^T + log_qw[key]) on the ACT engine (bias is per-partition
     = per-key in this orientation; 1024-wide spans amortize ACT overhead)
  4. accumulates [v^T | 1]^T @ P in PSUM -> numerator rows 0..63, denominator
     row 64 (the appended ones column), normalizes with a reciprocal +
     partition-broadcast multiply (the broadcast/store work is deferred two
     pipeline steps into the next chunk so the PE never waits on the DVE)
  5. after each 1024-query chunk qc, AllToAll reshards that chunk's outputs
     (dest core c gets queries [1024*qc+128*c, +128)), overlapping the
     collective with the next chunk's attention compute
  6. applies the output projection p_w per received 128-token chunk
The host packs weights (bf16) and constants to minimize DMA count, folds the
v bias into the output-projection bias (softmax weights sum to 1), and
scatters the per-core 128-token chunks into the full output.

Softmax skips max-subtraction: logits are q.k/8 + log(quad weights) which is
bounded well inside fp32 exp range.
"""

import contextlib
import sys
import types

import numpy as np

import concourse.bass as bass
import concourse.bacc as bacc
import concourse.tile as tile
from concourse import mybir
from concourse import bass_utils

# This container has no axon NTFF profile hook; shim the module so
# run_bass_kernel_spmd(trace=True) degrades gracefully instead of raising.
try:  # pragma: no cover
    import antenv.axon_hooks  # noqa: F401
except Exception:  # ModuleNotFoundError, or antenv missing entirely
    try:
        import antenv  # noqa: F401
    except Exception:
        antenv_mod = types.ModuleType("antenv")
        sys.modules["antenv"] = antenv_mod
    shim = types.ModuleType("antenv.axon_hooks")
    shim.get_axon_ntff_profile_hook = lambda: None
    sys.modules["antenv.axon_hooks"] = shim

F32 = mybir.dt.float32
F32R = mybir.dt.float32r
BF16 = mybir.dt.bfloat16
AF = mybir.ActivationFunctionType

C = 512          # channels
T = 4096         # tokens (H*W)
HD = 64          # head dim
NCORES = 8
NKT = T // 128   # 32 key tiles of 128
QC = 1024        # query chunk width for the attention inner loop
NQC = T // QC    # 4
CT = T // NCORES  # 512 tokens per core in the output projection
CHK = QC // NCORES  # 128: tokens per core per a2a chunk
SCALE = 1.0 / float(np.sqrt(HD))

_CACHE = {}
_VARIANT = "full"   # "full" | "notail" (skip a2a + output projection; debug/timing)


def _emit_body(nc, tc, io, rep):
    """Emit one full forward pass. `io` holds the DRAM tensor handles.

    Emission order software-pipelines the attention inner loop: the S^T
    matmuls run two iterations ahead of exp/AV so the PE fills S(kt+2)
    while ACT computes exp(kt), breaking the exp->AV->S->exp serial chain.
    k/v projections are interleaved into the qc=0 attention iterations
    (every key tile is consumed there); q projections are deferred to the
    query chunk that uses them, where the PE has slack under the ACT exp.
    """
    x, wqkv, wp, cst, onesr, y = io
    with contextlib.ExitStack() as ctx:
        big = ctx.enter_context(tc.tile_pool(name=f"big{rep}", bufs=1))
        wts = ctx.enter_context(tc.tile_pool(name=f"wts{rep}", bufs=1))
        vtp = ctx.enter_context(tc.tile_pool(name=f"vtp{rep}", bufs=1))
        ptlp = ctx.enter_context(tc.tile_pool(name=f"ptl{rep}", bufs=6))
        sml = ctx.enter_context(tc.tile_pool(name=f"sml{rep}", bufs=2))
        drp = ctx.enter_context(tc.tile_pool(name=f"drp{rep}", bufs=1, space="DRAM"))

        ps_stack = contextlib.ExitStack()
        # PSUM budget (8 banks of 2KB): S staging 2x2 banks, projection /
        # epilogue scratch 2x1 bank, AV accumulator 2 banks
        pss = ps_stack.enter_context(
            tc.tile_pool(name=f"pss{rep}", bufs=2, space="PSUM"))
        pjp = ps_stack.enter_context(
            tc.tile_pool(name=f"pjp{rep}", bufs=2, space="PSUM"))
        psa = ps_stack.enter_context(
            tc.tile_pool(name=f"psa{rep}", bufs=1, space="PSUM"))

        # ---- weight/const loads (packed: one DMA each) ------------------
        wqkv_sb = wts.tile([128, 4, 3, HD], BF16, tag="wqkv")
        nc.sync.dma_start(
            out=wqkv_sb[:, :, :, :],
            in_=wqkv[:, :].rearrange("(ci p) d -> p ci d", ci=4))
        wq_sb = wqkv_sb[:, :, 0, :]
        wk_sb = wqkv_sb[:, :, 1, :]
        wv_sb = wqkv_sb[:, :, 2, :]
        # cst cols: 0:32 log-quad bias per key tile, 32:36 output bias (pb),
        # 36 q bias (rows 0:64)
        cst_sb = wts.tile([128, 37], F32, tag="cst")
        nc.sync.dma_start(out=cst_sb, in_=cst[:, :])
        lqw_sb = cst_sb[:, 0:NKT]
        bq_sb = cst_sb[0:HD, 36:37]
        wp_sb = wts.tile([128, 4, C], BF16, tag="wp")
        onesr_sb = wts.tile([1, HD], F32R, tag="onesr")
        # f32r memset fails the ISA check in codegen; DMA the ones in
        nc.sync.dma_start(out=onesr_sb, in_=onesr[:, :])

        # ---- x loads, 1024-token groups; group 0 split so the 0:512
        # halves (needed by the prologue projections) land first
        x_sb = big.tile([128, 4, T], BF16, tag="x")
        def load_x_group(g, nsplit=2):
            w = 1024 // nsplit
            for s in range(nsplit):
                c0 = 1024 * g + w * s
                nc.sync.dma_start(
                    out=x_sb[:, :, c0:c0 + w],
                    in_=x[:, c0:c0 + w].rearrange("(ci p) t -> p ci t", ci=4))
        load_x_group(0, nsplit=2)

        q_dup = big.tile([128, T], BF16, tag="qd")
        k_dup = big.tile([128, T], BF16, tag="kd")
        vt = [vtp.tile([128, HD + 1], BF16, tag=f"vt{t}", name=f"vt{t}")
              for t in range(NKT)]

        def emit_qk_chunk(w_sb, b_sb, dst, n, eng=None):
            # channel-major projection of 512 tokens, written twice (rows
            # 0:64 and 64:128) so S^T matmuls can row-pair two query
            # subchunks without serializing on one PE weight-row range.
            # The k bias is dropped entirely — it only contributes a
            # per-query constant q.bk to the logits, which softmax cancels.
            # (gpsimd cannot read PSUM on real hardware, so movers stay DVE.)
            eng = eng or nc.vector
            ps = pjp.tile([HD, 512], F32, tag="pj")
            for ci in range(4):
                nc.tensor.matmul(ps, w_sb[:, ci, :],
                                 x_sb[:, ci, 512 * n:512 * (n + 1)],
                                 start=(ci == 0), stop=(ci == 3))
            sl = slice(512 * n, 512 * (n + 1))
            if b_sb is None:
                eng.tensor_copy(out=dst[0:HD, sl], in_=ps)
            else:
                eng.tensor_scalar_add(out=dst[0:HD, sl], in0=ps, scalar1=b_sb)
            nc.sync.dma_start(out=dst[HD:128, sl], in_=dst[0:HD, sl])

        def emit_vt(t, eng=None):
            # token-major v^T tile with memset ones column (denominator);
            # bf16 keeps the 64-wide output at full PE rate
            eng = eng or nc.vector
            ps = pjp.tile([128, HD], F32, tag="pj")
            for ci in range(4):
                nc.tensor.matmul(ps, x_sb[:, ci, 128 * t:128 * (t + 1)],
                                 wv_sb[:, ci, :],
                                 start=(ci == 0), stop=(ci == 3))
            nc.vector.memset(vt[t][:, HD:HD + 1], 1.0)
            eng.tensor_copy(out=vt[t][:, 0:HD], in_=ps)

        # prologue: exactly the projections the first iterations need (the
        # S^T row-pairing means S(0,0) already reads q chunks 0 and 1);
        # these movers go on the DVE, which is otherwise empty this early
        emit_qk_chunk(wq_sb, bq_sb, q_dup, 0, eng=nc.vector)
        emit_qk_chunk(wq_sb, bq_sb, q_dup, 1, eng=nc.vector)
        emit_qk_chunk(wk_sb, None, k_dup, 0, eng=nc.vector)

        # ---- attention (flat software pipeline over (qc, kt)) ----------
        oh = big.tile([HD, T], BF16, tag="oh")
        snd = [drp.tile([NCORES, HD, CHK], BF16, tag=f"snd{g}", name=f"snd{g}")
               for g in range(NQC)]
        rcv = [drp.tile([NCORES, HD, CHK], BF16, tag=f"rcv{g}", name=f"rcv{g}")
               for g in range(NQC)]
        at = [big.tile([128, 4, CHK], BF16, tag=f"at{g}", name=f"at{g}")
              for g in range(NQC)]

        ss_tiles = {}

        def emit_s(qc, kt, dup=True):
            ss = pss.tile([128, QC], F32, tag="ss")
            ss_tiles[(qc, kt)] = ss
            for sub in range(2):
                b0 = 64 * sub if dup else 0
                qoff = QC * qc + 512 * sub
                nc.tensor.matmul(ss[:, 512 * sub:512 * (sub + 1)],
                                 k_dup[b0:b0 + 64, 128 * kt:128 * (kt + 1)],
                                 q_dup[b0:b0 + 64, qoff:qoff + 512],
                                 start=True, stop=True)

        def emit_finish(qc, src, rcp):
            # softmax epilogue for chunk qc: broadcast 1/den across 64
            # partitions with K=1 matmuls into one-bank scratch tiles,
            # normalize, reshard; per-512 halves so the first snd slice
            # leaves while the second half still normalizes
            for sub in range(2):
                rb = pjp.tile([HD, 512], F32, tag="pj", name=f"rb{qc}_{sub}")
                nc.tensor.matmul(rb, onesr_sb,
                                 rcp[:, 512 * sub:512 * (sub + 1)],
                                 start=True, stop=True)
                osl = slice(QC * qc + 512 * sub, QC * qc + 512 * (sub + 1))
                nc.vector.tensor_mul(out=oh[:, osl],
                                     in0=src[0:HD, 512 * sub:512 * (sub + 1)],
                                     in1=rb)
                if _VARIANT != "notail":
                    # dest core c takes queries [QC*qc + CHK*c, +CHK); the
                    # SBUF side must stay partition-major, so the dest-major
                    # transposition lives in the DRAM-side access pattern
                    nc.sync.dma_start(
                        out=snd[qc][4 * sub:4 * (sub + 1), :, :]
                            .rearrange("c h j -> h c j"),
                        in_=oh[:, osl].rearrange("h (c j) -> h c j", c=4))
            if _VARIANT == "notail":
                return
            nc.gpsimd.collective_compute(
                "AllToAll", mybir.AluOpType.bypass,
                replica_groups=[list(range(NCORES))],
                ins=[snd[qc][:, :, :]], outs=[rcv[qc][:, :, :]])

        yo_tiles = {}

        def emit_outproj_at(gq, tail=False):
            # fetch the received 128-token chunk gq from DRAM; mid-run this
            # rides the gpsimd queue so its wait on the collective cannot
            # block the sync-queue DMA stream
            dma = nc.sync.dma_start if tail else nc.gpsimd.dma_start
            dma(out=at[gq][:, :, :],
                in_=rcv[gq][:, :, :].rearrange("a b c -> (a b) c")
                    .rearrange("(ci p) c -> p ci c", ci=4))
            yo_tiles[gq] = sml.tile([128, 4, CHK], F32, tag="yo",
                                    name=f"yo{gq}")

        def emit_outproj_m(gq, m, tail=False):
            # one 128-channel slab of chunk gq's output projection; slabs are
            # spread across the owning qc so the PE absorbs them in the slack
            # under the ACT exp stream
            ps = pjp.tile([128, CHK], F32, tag="pj", name=f"yps{gq}_{m}")
            for ci in range(4):
                nc.tensor.matmul(ps, wp_sb[:, ci, 128 * m:128 * (m + 1)],
                                 at[gq][:, ci, :],
                                 start=(ci == 0), stop=(ci == 3))
            nc.vector.tensor_scalar_add(out=yo_tiles[gq][:, m, :], in0=ps,
                                        scalar1=cst_sb[:, 32 + m:33 + m])
            if m == 3:
                # always the sync queue: on the gpsimd queue this wait for
                # the slab biases would head-block a later collective
                nc.sync.dma_start(
                    out=y[:, CHK * gq:CHK * (gq + 1)].rearrange(
                        "(m p) c -> p m c", m=4),
                    in_=yo_tiles[gq][:, :, :])

        def emit_outproj(gq, tail=False):
            emit_outproj_at(gq, tail)
            for m in range(4):
                emit_outproj_m(gq, m, tail)

        # interleaved projection/load work, keyed by global pipeline step.
        # During qc=0 we still owe the whole key side: k chunks 1..7 (chunk n
        # feeds kt 4n..4n+3) and vt 2..31 (tile t feeds step t), plus the x
        # token-group loads. q chunks land in the qc before their first use,
        # where the PE has slack under the ACT exp stream.
        prefetch = {}
        for i in range(1, 4):
            prefetch.setdefault(6 * i - 4, []).append(("xg", i))
        for n in range(1, 8):
            # chunk n feeds S tiles from step 4n; leave slack for the
            # row-duplicate DMA behind the projection
            prefetch.setdefault(max(0, 4 * n - 5), []).append(("k", n))
        for t in range(2, NKT):
            prefetch.setdefault(t - 1, []).append(("vt", t))
        for n in range(2, 8):
            # chunk n first used at qc n//2 (step 32*(n//2)); emit at the
            # START of the preceding qc — the PE runs in emission order, so
            # an early slot keeps the projection (and the DVE bias-add
            # behind it) from head-blocking the DVE queue near boundaries
            prefetch.setdefault(32 * (n // 2 - 1) + 4 + 4 * (n % 2), []).append(("q", n))
        prefetch.setdefault(20, []).append(("wp",))

        steps = [(qc, kt) for qc in range(NQC) for kt in range(NKT)]
        av_tiles = {}
        finish_q = []
        # the first two S tiles read rows 0:64 for both subchunks so the
        # first exp is not gated on the row-64:128 duplicate writes
        emit_s(*steps[0], dup=False)
        emit_s(*steps[1], dup=False)
        emit_vt(0)
        emit_vt(1)
        for g, (qc, kt) in enumerate(steps):
            for item in prefetch.get(g, ()):
                if item[0] == "xg":
                    load_x_group(item[1])
                elif item[0] == "k":
                    emit_qk_chunk(wk_sb, None, k_dup, item[1])
                elif item[0] == "q":
                    emit_qk_chunk(wq_sb, bq_sb, q_dup, item[1])
                elif item[0] == "vt":
                    emit_vt(item[1])
                elif item[0] == "wp":
                    # gpsimd queue: keeps this large transfer's issue path
                    # off the latency-sensitive sync-queue DMA stream
                    nc.gpsimd.dma_start(
                        out=wp_sb[:, :, :],
                        in_=wp[:, :].rearrange("(ci p) d -> p ci d", ci=4))
            if kt == 2 and finish_q:
                emit_finish(*finish_q.pop())
            if _VARIANT != "notail":
                # gpsimd-queue order is the contract here: the Pool SEQ
                # blocks on its head instruction's waits, so each at-fetch is
                # emitted right AFTER the next collective; chunk qc-2's
                # projection slabs spread over this qc in the PE slack
                if kt == 4 and qc >= 2:
                    emit_outproj_at(qc - 2)
                elif kt == 24 and qc == NQC - 1:
                    # last mid-run chunk: fetched before the tail collective
                    # is queued, so its slabs overlap that collective
                    emit_outproj_at(qc - 1)
                elif qc >= 2 and kt in (10, 14, 18, 22):
                    emit_outproj_m(qc - 2, (kt - 10) // 4)
            if kt == 0:
                av_tiles[qc] = psa.tile([HD + 1, QC], F32, tag="av",
                                        name=f"av{qc}")
            av = av_tiles[qc]
            ss = ss_tiles.pop((qc, kt))
            pt = ptlp.tile([128, QC], BF16, tag="pt")
            nc.scalar.activation(out=pt, in_=ss, func=AF.Exp,
                                 scale=SCALE, bias=lqw_sb[:, kt:kt + 1])
            if g + 2 < len(steps):
                emit_s(*steps[g + 2])
            for sub in range(2):
                nc.tensor.matmul(av[:, 512 * sub:512 * (sub + 1)],
                                 vt[kt], pt[:, 512 * sub:512 * (sub + 1)],
                                 start=(kt == 0), stop=(kt == NKT - 1),
                                 skip_group_check=True)
            if kt == NKT - 1:
                # copy the finished accumulator out of PSUM (rows 0..63
                # numerator, row 64 denominator; the normalize multiply may
                # read at most one PSUM operand, and the broadcast tile
                # already is one). For qc<3 the remaining epilogue is
                # deferred two steps into the next chunk; the last chunk
                # finishes inline.
                av_sb = sml.tile([HD + 1, QC], F32, tag="avs")
                nc.vector.tensor_copy(out=av_sb, in_=av)
                rcp = sml.tile([1, QC], F32R, tag="rcp")
                with nc.allow_low_precision(
                        reason="1/den broadcast via f32r matmul; f32r "
                               "keeps ~19 mantissa bits, fine here"):
                    nc.vector.reciprocal(out=rcp, in_=av_sb[HD:HD + 1, :])
                if qc < NQC - 1:
                    finish_q.append((qc, av_sb, rcp))
                else:
                    emit_finish(qc, av_sb, rcp)
        if _VARIANT == "notail":
            ps_stack.close()
            nc.gpsimd.dma_start(out=y[0:HD, :], in_=oh[:, 0:CT])
            return
        # tail: chunk 2's slabs overlap the last collective (its at-DMA was
        # emitted at qc3 kt24, before the collective); then chunk 3 takes
        # the sync-queue paths
        for m in range(4):
            emit_outproj_m(NQC - 2, m, tail=True)
        emit_outproj_at(NQC - 1, tail=True)
        for m in range(4):
            emit_outproj_m(NQC - 1, m, tail=True)
        ps_stack.close()


def _build(repeat=1):
    nc = bacc.Bacc("TRN2", target_bir_lowering=False, debug=False,
                   num_devices=NCORES)
    x = nc.dram_tensor("x", [C, T], BF16, kind="ExternalInput")
    wqkv = nc.dram_tensor("wqkv", [C, 3 * HD], BF16, kind="ExternalInput")
    wp = nc.dram_tensor("wp", [C, C], BF16, kind="ExternalInput")
    cst = nc.dram_tensor("cst", [128, 37], F32, kind="ExternalInput")
    onesr = nc.dram_tensor("onesr", [1, HD], F32R, kind="ExternalInput")
    y = nc.dram_tensor("y", [C, CT], F32, kind="ExternalOutput")
    io = (x, wqkv, wp, cst, onesr, y)

    with tile.TileContext(nc) as tc:
        for rep in range(repeat):
            _emit_body(nc, tc, io, rep)

    nc.finalize()
    return nc


def _get_nc(repeat=1):
    key = ("nc", repeat)
    if key not in _CACHE:
        _CACHE[key] = _build(repeat)
    return _CACHE[key]


def _bf16(a):
    import ml_dtypes
    return np.ascontiguousarray(np.asarray(a, np.float32).astype(ml_dtypes.bfloat16))


def _in_maps(query, q_w, q_b, k_w, k_b, v_w, v_b, p_w, p_b, log_quad_weights):
    x = _bf16(np.asarray(query, np.float32).reshape(C, T))
    wp = _bf16(np.asarray(p_w, np.float32).T)
    # v bias folds into the projection bias: softmax weights sum to 1, so
    # out = P(v + b_v)/den = Pv/den + b_v and y += p_w @ b_v
    pb_full = (np.asarray(p_b, np.float32)
               + np.asarray(p_w, np.float32) @ np.asarray(v_b, np.float32))
    lqw = np.asarray(log_quad_weights, np.float32).reshape(NKT, 128).T
    maps = []
    for h in range(NCORES):
        hs = slice(HD * h, HD * (h + 1))
        # k_b is dropped on purpose: it only adds a per-query constant q.bk
        # to the logits, which the softmax normalization cancels exactly
        cst = np.zeros((128, 37), np.float32)
        cst[:, 0:NKT] = lqw
        cst[:, NKT:NKT + 4] = pb_full.reshape(4, 128).T
        cst[0:HD, 36] = np.asarray(q_b, np.float32)[hs]
        wqkv = np.concatenate([
            np.asarray(q_w, np.float32)[hs, :].T,
            np.asarray(k_w, np.float32)[hs, :].T,
            np.asarray(v_w, np.float32)[hs, :].T], axis=1)
        maps.append(dict(
            x=x,
            wqkv=_bf16(wqkv),
            wp=wp,
            cst=np.ascontiguousarray(cst),
            onesr=np.ones((1, HD), np.float32),
        ))
    return maps


def _run(in_maps, repeat=1, **kw):
    nc = _get_nc(repeat)
    return bass_utils.run_bass_kernel_spmd(nc, in_maps, list(range(NCORES)), **kw)


def _assemble(results):
    # core c's y column 128*g + j holds global token 1024*g + 128*c + j
    full = np.empty((C, T), np.float32)
    for c in range(NCORES):
        yc = results[c]["y"]
        for g in range(NQC):
            full[:, QC * g + CHK * c:QC * g + CHK * (c + 1)] = \
                yc[:, CHK * g:CHK * (g + 1)]
    return np.ascontiguousarray(full.reshape(1, C, 64, 64).astype(np.float32))


def kernel(**inputs):
    res = _run(_in_maps(**inputs))
    return _assemble(res.results)
